# revision 2
# baseline (speedup 1.0000x reference)
"""DFFN Trainium2 kernel for nn_DFFN_81535659147929.

Pipeline: project_in (1x1 conv, 64->340) -> per-8x8-patch rFFT2 * learned
filter -> irFFT2 -> depthwise 3x3 conv -> GELU gate -> project_out (170->64).

Key algebra: the per-patch rFFT2*w->irFFT2 step is, per channel c, a linear
map M_c on the 64 patch pixels, and all M_c are simultaneously diagonalized
by the (channel-independent) orthonormal real 2D-DFT basis C:
M_c = C^T diag(lam_c) C.  So the whole FFT stage becomes two shared-weight
matmuls around a per-(channel,freq) scale.

Sharding: data-parallel, core = (image b = core//2, row half core%2) with an
8-row patch-aligned halo; weights replicated.

Device pipeline per core:
  Phase 1, per 2-patch block (128 pixels on partitions):
    A:   psumA[pix,col] = x_block^T @ w_inT     (project_in, K=64)
    C:   psumB[frq,col] = CC @ sA               (forward transform)
    lam: sB = psumB * lam_tile                  (DVE, psum evac)
    E:   psumZ[col,pix] = sB_chunk^T @ CCrhs    (inverse transform; 4 blocks
         share a PSUM bank, one strided evac per bank -> spatial z bands,
         DMA'd to DRAM zbufs)
  Phase 2 (dwconv+gate), per (16-channel group, 8-row quad):
    partitions = (16 ch, 8 rows); the 3x3 conv becomes 3 column-shifted
    matmuls with a banded row-coupling lhsT (dense, not diagonal), psum-
    accumulated; 8 input rows yield 6 output rows (quads overlap by 2).
    Channel groups alternate x1/x2 so the GELU gate pairs align on
    partitions across the two psum tiles.  g bounced to DRAM.
  Phase 3: project_out (K=176 over two chunks) per 16-row band, DMA out.

Channel column order (NCOL=352): 22 interleaved groups of 16,
group 2m = x1 channels 16m..16m+15, group 2m+1 = x2 (= +170), last pair
padded with zero channels.
"""

import numpy as np
import ml_dtypes

import concourse.bass as bass
import concourse.mybir as mybir
from concourse import bacc, tile
from concourse.bass_utils import run_bass_kernel_spmd

BF16 = mybir.dt.bfloat16
F32 = mybir.dt.float32

DIM = 64
C2 = 340
P = 8
B, H, W = 4, 256, 256
N_CORES = 8
ROWS = H // 2
HALO = P
NCOL = 352          # 22 groups x 16, incl 12 pad channels
NGRP = 22
GCH = 176           # padded gate channels (11 groups x 16)
QSTEP = 6           # output rows per dwconv quad (8 input rows)

_cache = {}


# ----------------------------------------------------------------- host math

def _build_basis():
    rows = []
    seen = set()
    p1, p2 = np.meshgrid(np.arange(P), np.arange(P), indexing="ij")
    for u in range(P):
        for v in range(P):
            if (u, v) in seen:
                continue
            nu, nv = (-u) % P, (-v) % P
            th = 2 * np.pi * (u * p1 + v * p2) / P
            if (nu, nv) == (u, v):
                rows.append((np.cos(th) / 8.0).ravel())
            else:
                seen.add((nu, nv))
                rows.append((np.sqrt(2) / 8.0) * np.cos(th).ravel())
                rows.append((np.sqrt(2) / 8.0) * np.sin(th).ravel())
            seen.add((u, v))
    return np.array(rows, dtype=np.float64)


def _lam_for(fft_w, C):
    basis = C.reshape(64, P, P)
    F = np.fft.rfft2(basis)
    w = fft_w.reshape(C2, 1, P, P // 2 + 1).astype(np.float64)
    r = np.fft.irfft2(F[None] * w, s=(P, P))
    return np.einsum('kpq,ckpq->ck', basis, r)      # [C2, 64]


def _col_to_c2():
    """col (0..351) -> c2 channel or -1 (pad)."""
    cols = np.full(NCOL, -1, np.int64)
    for g in range(NGRP):
        m, half = divmod(g, 2)
        for j in range(16):
            ch = 16 * m + j
            if ch < 170:
                cols[g * 16 + j] = ch + 170 * half
    return cols


def _pix_maps():
    C = _build_basis()
    CCrhs = np.zeros((128, 128))
    for pc2 in range(2):
        for f in range(64):
            for p1 in range(P):
                for p2 in range(P):
                    k = p1 * 16 + pc2 * 8 + p2
                    CCrhs[pc2 * 64 + f, k] = C[f, p1 * 8 + p2]
    return CCrhs.T.copy(), CCrhs, C


def _prep_weights(w_in, w_dw, fft_w, w_out):
    CClhsT, CCrhs, C = _pix_maps()
    lam = _lam_for(fft_w, C)
    cols = _col_to_c2()
    valid = cols >= 0

    w_inT = np.zeros((64, NCOL))
    w_inT[:, valid] = w_in.T[:, cols[valid]]

    lam_t = np.zeros((128, NCOL))
    lam_sel = np.zeros((NCOL, 64))
    lam_sel[valid] = lam[cols[valid]]
    lam_t[:] = np.tile(lam_sel.T, (2, 1))[:128]

    dw = w_dw.reshape(C2, 3, 3)
    # dwconv banded lhsT per (group, dx): [128, 128] blockdiag over 16 ch of
    # T[r_in, o] = w_dw[c, r_in - o, dx], r_in - o in {0,1,2}
    dd = np.zeros((128, NGRP * 3 * 128))
    for g in range(NGRP):
        for dx in range(3):
            blk = np.zeros((128, 128))
            for j in range(16):
                c2 = cols[g * 16 + j]
                if c2 < 0:
                    continue
                for o in range(8):
                    for dy in range(3):
                        ri = o + dy
                        if ri < 8:
                            blk[j * 8 + ri, j * 8 + o] = dw[c2, dy, dx]
            dd[:, (g * 3 + dx) * 128:(g * 3 + dx + 1) * 128] = blk

    # project_out lhsT: cols 0:64 = gate-ch 0..127, cols 64:128 = 128..175
    wo = np.zeros((128, 128))
    wo[0:128, 0:64] = w_out.T[0:128]
    wo[0:42, 64:128] = w_out.T[128:170]
    bf = ml_dtypes.bfloat16
    return {
        "w_inT": w_inT.astype(bf),
        "cclhsT": CClhsT.astype(bf),
        "ccrhs": CCrhs.astype(bf),
        "lam_t": lam_t.astype(bf),
        "dd": dd.astype(bf),
        "wo": wo.astype(bf),
    }


# ---------------------------------------------------------------- bass build

def build_nc(rows=ROWS, dbg=False):
    rh = rows + 2 * HALO
    npr = rh // P
    nband = rows // 16
    nquad = (rows + QSTEP - 1) // QSTEP          # 22 for rows=128
    assert rows % 16 == 0

    nc = bacc.Bacc("TRN2", target_bir_lowering=False, debug=False,
                   num_devices=N_CORES)
    x_d = nc.dram_tensor("x", [DIM, rh * W], BF16, kind="ExternalInput")
    winT_d = nc.dram_tensor("w_inT", [64, NCOL], BF16, kind="ExternalInput")
    cclhsT_d = nc.dram_tensor("cclhsT", [128, 128], BF16, kind="ExternalInput")
    ccrhs_d = nc.dram_tensor("ccrhs", [128, 128], BF16, kind="ExternalInput")
    lam_d = nc.dram_tensor("lam_t", [128, NCOL], BF16, kind="ExternalInput")
    dd_d = nc.dram_tensor("dd", [128, NGRP * 3 * 128], BF16,
                          kind="ExternalInput")
    wo_d = nc.dram_tensor("wo", [128, 128], BF16, kind="ExternalInput")
    out_d = nc.dram_tensor("out", [DIM, rows * W], F32, kind="ExternalOutput")

    # z bounce buffers, one per 128-col chunk of the NCOL channel columns
    zkind = "ExternalOutput" if dbg else "Internal"
    zbufs = [nc.dram_tensor(f"zbuf{i}", [128, rh * W], BF16, kind=zkind)
             for i in range(3)]          # chunks: 128, 128, 96 rows used
    gbuf_d = nc.dram_tensor("gbuf", [128, 11 * nquad * W], BF16, kind=zkind)

    G = mybir.ActivationFunctionType.Gelu
    chunks = [(0, 128), (128, 128), (256, 96)]

    with tile.TileContext(nc) as tc:
        with tc.tile_pool(name="consts", bufs=1) as cpool:
            w_inT = cpool.tile([64, NCOL], BF16)
            nc.sync.dma_start(out=w_inT[:], in_=winT_d[:])
            cclhsT = cpool.tile([128, 128], BF16)
            nc.sync.dma_start(out=cclhsT[:], in_=cclhsT_d[:])
            ccrhs = cpool.tile([128, 128], BF16)
            nc.sync.dma_start(out=ccrhs[:], in_=ccrhs_d[:])
            lam_t = cpool.tile([128, NCOL], BF16)
            nc.sync.dma_start(out=lam_t[:], in_=lam_d[:])
            dd = cpool.tile([128, NGRP * 3 * 128], BF16)
            nc.sync.dma_start(out=dd[:], in_=dd_d[:])
            wo = cpool.tile([128, 128], BF16)
            nc.sync.dma_start(out=wo[:], in_=wo_d[:])

            # ---------------- all pools coexist: PSUM adds up to 8 banks
            dxs = [(1, 0, 0, 256), (0, 0, 1, 255), (2, 1, 0, 255)]
            gq = gbuf_d[:].rearrange("(j o) (m q w) -> j m q o w",
                                     j=16, o=8, m=11, q=nquad, w=W)
            npair = (nquad + 1) // 2
            with (
                tc.tile_pool(name="p1x", bufs=1) as xpool,
                tc.tile_pool(name="p1s", bufs=4) as spool,
                tc.tile_pool(name="p1sb", bufs=8) as sbpool,
                tc.tile_pool(name="p1z", bufs=2) as zpool,
                tc.tile_pool(name="p1ps", bufs=2, space="PSUM") as pspool,
                tc.tile_pool(name="p1pz", bufs=2, space="PSUM") as pzpool,
                tc.tile_pool(name="p2z", bufs=4) as zqpool,
                tc.tile_pool(name="p2g", bufs=4) as gpool,
                tc.tile_pool(name="p2ps", bufs=1, space="PSUM") as qpool,
                tc.tile_pool(name="p3g", bufs=2) as grpool,
                tc.tile_pool(name="p3o", bufs=1) as opool,
                tc.tile_pool(name="p3ps", bufs=2, space="PSUM") as popool,
            ):
                # x arrives host-patchified: [64, (pr, pcp, p1, pc2, p2)]
                x_sb = xpool.tile([64, rh * W], BF16)
                bsz = 32 * 128      # one 2-patch-row band of columns
                for bb in range(npr // 2):
                    nc.sync.dma_start(out=x_sb[:, bb * bsz:(bb + 1) * bsz],
                                      in_=x_d[:, bb * bsz:(bb + 1) * bsz])

                # HAM warmup: dense dummy matmuls while the first DMAs land
                wps = pspool.tile([128, NCOL], F32, tag="pAB", name="warm")
                for _ in range(40):
                    nc.tensor.matmul(wps[:, 0:128], cclhsT[:], ccrhs[:],
                                     start=True, stop=True)

                def emit_band(band):
                    prs = [2 * band, 2 * band + 1]
                    zbs = [zpool.tile([128, 16 * W], BF16, tag=f"zb{i}",
                                      name=f"zb{i}")
                           for i in range(3)]
                    for bi, pr in enumerate(prs):
                        for pq in range(4):          # 4 blocks of 4 pcp
                            sBs = []
                            for pj in range(4):
                                pcp = pq * 4 + pj
                                psA = pspool.tile([128, NCOL], F32, tag="pAB")
                                xblk = x_sb[:, (pr * 16 + pcp) * 128:
                                            (pr * 16 + pcp + 1) * 128]
                                nc.tensor.matmul(psA[:], xblk, w_inT[:],
                                                 start=True, stop=True)
                                sA = spool.tile([128, NCOL], BF16, tag="sA")
                                if pj % 2 == 0:
                                    nc.scalar.copy(sA[:], psA[:])
                                else:
                                    nc.vector.tensor_copy(sA[:], psA[:])
                                psB = pspool.tile([128, NCOL], F32, tag="pAB",
                                                  name="psB")
                                nc.tensor.matmul(psB[:], cclhsT[:], sA[:],
                                                 start=True, stop=True)
                                sB = sbpool.tile([128, NCOL], BF16, tag="sB")
                                nc.vector.tensor_mul(sB[:], psB[:], lam_t[:])
                                sBs.append(sB)
                            for ci, (c0, m) in enumerate(chunks):
                                psZ = pzpool.tile([128, 512], F32,
                                                  tag="psZ")
                                for pj in range(4):
                                    nc.tensor.matmul(
                                        psZ[0:m, pj * 128:(pj + 1) * 128],
                                        sBs[pj][:, c0:c0 + m], ccrhs[:],
                                        start=(pj == 0), stop=(pj == 3))
                                # evac: psum cols (pj, p1, pc2p2) read as
                                # (p1, pj, pc2p2) -> zband rows/cols
                                src = psZ[:].rearrange(
                                    "c (pj p1 q) -> c p1 pj q",
                                    pj=4, p1=8, q=16)[0:m]
                                dstr = zbs[ci][:].rearrange(
                                    "c (r pcp q) -> c r pcp q",
                                    r=16, pcp=16, q=16)
                                dst = dstr[0:m, bi * 8:bi * 8 + 8,
                                           pq * 4:pq * 4 + 4, :]
                                if (pq + ci) % 2 == 0:
                                    nc.scalar.copy(dst, src)
                                else:
                                    nc.vector.tensor_copy(dst, src)
                    r0 = 2 * band * 8 * W
                    for i in range(3):
                        nc.sync.dma_start(out=zbufs[i][:, r0:r0 + 16 * W],
                                          in_=zbs[i][:])

                # ------ phases 2+3 interleaved over quad-pair groups
                def emit_qg(qg):
                    q0 = 2 * qg
                    quads = [q for q in (q0, q0 + 1) if q < nquad]
                    nq = len(quads)
                    for m in range(11):              # gate pair m
                        pss = []
                        for half in range(2):
                            g = 2 * m + half
                            ci, roff = g // 8, (g % 8) * 16
                            zt2 = zqpool.tile([128, 2 * W], BF16,
                                              tag=f"zq{half}",
                                              name=f"zq{half}")
                            zsrc = zbufs[ci][:].rearrange(
                                "c (r w) -> c r w", r=rh, w=W)
                            eng = nc.gpsimd
                            for qi, q in enumerate(quads):
                                eng.dma_start(
                                    out=zt2[:, qi * W:(qi + 1) * W],
                                    in_=zsrc[roff:roff + 16,
                                             QSTEP * q + 7:QSTEP * q + 15, :])
                            ps = qpool.tile([128, 2 * W], F32,
                                            tag=f"ps{half}", name=f"ps{half}")
                            zr = zt2[:].rearrange("c (q w) -> c q w", q=2, w=W)
                            pr = ps[:].rearrange("c (q w) -> c q w", q=2, w=W)
                            for k, (dx, wi0, wo0, wn) in enumerate(dxs):
                                lhs = dd[:, (g * 3 + dx) * 128:
                                         (g * 3 + dx + 1) * 128]
                                nc.tensor.matmul(
                                    pr[:, 0:nq, wo0:wo0 + wn], lhs,
                                    zr[:, 0:nq, wi0:wi0 + wn],
                                    start=(k == 0), stop=(k == 2))
                            pss.append(ps)
                        ge = gpool.tile([128, 2 * W], BF16, tag="ge")
                        nc.scalar.activation(ge[:, 0:nq * W],
                                             pss[0][:, 0:nq * W], G)
                        gt2 = gpool.tile([128, 2 * W], BF16, tag="gt2")
                        nc.vector.tensor_mul(gt2[:, 0:nq * W],
                                             ge[:, 0:nq * W],
                                             pss[1][:, 0:nq * W])
                        nc.sync.dma_start(
                            out=gbuf_d[:, (m * nquad + q0) * W:
                                       (m * nquad + q0 + nq) * W],
                            in_=gt2[:, 0:nq * W])

                    # ---- phase 3 for this quad pair
                    ncols = nq * QSTEP * W
                    nrows = min(nq * QSTEP, rows - QSTEP * q0)
                    gA = grpool.tile([128, 2 * QSTEP * W], BF16, tag="gA")
                    gB = grpool.tile([48, 2 * QSTEP * W], BF16, tag="gB")
                    for mm in range(8):
                        for qi in range(nq):
                            nc.sync.dma_start(
                                out=gA[16 * mm:16 * mm + 16,
                                       qi * QSTEP * W:(qi + 1) * QSTEP * W],
                                in_=gq[:, mm, q0 + qi, 0:QSTEP, :])
                    for mm in range(3):
                        for qi in range(nq):
                            nc.sync.dma_start(
                                out=gB[16 * mm:16 * mm + 16,
                                       qi * QSTEP * W:(qi + 1) * QSTEP * W],
                                in_=gq[:, 8 + mm, q0 + qi, 0:QSTEP, :])
                    obnd = opool.tile([64, 2 * QSTEP * W], F32, tag="oband")
                    nct = (ncols + 511) // 512
                    for ct in range(nct):
                        c0, c1 = ct * 512, min((ct + 1) * 512, ncols)
                        po = popool.tile([64, 512], F32, tag="po")
                        nc.tensor.matmul(po[:, 0:c1 - c0], wo[0:128, 0:64],
                                         gA[:, c0:c1],
                                         start=True, stop=False)
                        nc.tensor.matmul(po[:, 0:c1 - c0], wo[0:48, 64:128],
                                         gB[:, c0:c1],
                                         start=False, stop=True)
                        nc.vector.tensor_copy(obnd[:, c0:c1],
                                              po[:, 0:c1 - c0])
                    nc.sync.dma_start(
                        out=out_d[:, QSTEP * q0 * W:
                                  (QSTEP * q0 + nrows) * W],
                        in_=obnd[:, 0:nrows * W])

                # ------ interleaved emission: band b, then ready quad-pairs
                nbands = npr // 2
                ready = {}
                for k in range(npair):
                    quads_k = [q for q in (2 * k, 2 * k + 1) if q < nquad]
                    maxrow = QSTEP * quads_k[-1] + 14
                    b = min(nbands - 1, maxrow // 16)
                    ready.setdefault(b, []).append(k)
                done = set()
                for band in range(nbands):
                    emit_band(band)
                    for k in ready.get(band, []):
                        emit_qg(k)
                        done.add(k)
                for k in range(npair):
                    if k not in done:
                        emit_qg(k)

    nc.compile()
    return nc


# ----------------------------------------------------------------- interface

def _get_program(rows=ROWS):
    key = ("nc", rows)
    if key not in _cache:
        _cache[key] = build_nc(rows)
    return _cache[key]


def _patchify(xs):
    rh = xs.shape[1]
    xp = xs.reshape(DIM, rh // 8, 8, 16, 2, 8).transpose(0, 1, 3, 2, 4, 5)
    return np.ascontiguousarray(xp).reshape(DIM, rh * W).astype(
        ml_dtypes.bfloat16)


def _shard_x(x, rows=ROWS):
    rh = rows + 2 * HALO
    shards = []
    for c in range(N_CORES):
        b, hh = divmod(c, 2)
        r0 = hh * rows
        xs = np.zeros((DIM, rh, W), np.float32)
        lo, hi = r0 - HALO, r0 + rows + HALO
        slo, shi = max(lo, 0), min(hi, x.shape[2])
        xs[:, slo - lo:shi - lo] = x[b, :, slo:shi]
        shards.append(_patchify(xs))
    return shards


def _run(x, w_in, w_dw, fft_w, w_out, trace=False):
    nc = _get_program()
    wts = _prep_weights(np.asarray(w_in, np.float32),
                        np.asarray(w_dw, np.float32).reshape(C2, 3, 3),
                        np.asarray(fft_w, np.float32),
                        np.asarray(w_out, np.float32))
    shards = _shard_x(np.asarray(x, np.float32))
    in_maps = [{"x": s, **wts} for s in shards]
    res = run_bass_kernel_spmd(nc, in_maps, core_ids=list(range(N_CORES)),
                               trace=trace)
    out = np.zeros((B, DIM, H, W), np.float32)
    for c in range(N_CORES):
        b, hh = divmod(c, 2)
        out[b, :, hh * ROWS:(hh + 1) * ROWS] = (
            res.results[c]["out"].reshape(DIM, ROWS, W))
    return out, res.exec_time_ns


def kernel(x, w_in, w_dw, fft_w, w_out):
    out, _ = _run(x, w_in, w_dw, fft_w, w_out, trace=False)
    return out



# revision 4
# speedup vs baseline: 1.1956x; 1.1956x over previous
"""DFFN Trainium2 kernel v2 for nn_DFFN_81535659147929.

Pipeline: project_in (1x1 conv, 64->340) -> per-8x8-patch rFFT2 * learned
filter -> irFFT2 -> depthwise 3x3 conv -> GELU gate -> project_out (170->64).

v2 redesign vs baseline:
  - Stage A per 2-patch block: Fx = DFT(x) first (K=128 pix, out [64ch,128f]),
    then project_in in freq space (K=64, FD=352, rhs=w_inT const), then
    lambda multiply, then inverse per channel chunk.  Cheaper evacs.
  - z stays in SBUF in a 40-row ring (3 chunks x [128, 40*256] bf16);
    no DRAM bounce.
  - dwconv: 16-row windows (14 valid out rows), 8ch x 16row partition
    tiles, 3 col-shifted banded matmuls; gathers batched as ONE DMA per
    (chunk, window) [2MB] instead of per-subgroup.
  - gate (gelu(x1)*x2) right out of psum; restructure to [gate-ch, pix]
    via one SWDGE DMA per (pair, window); project_out per window from
    SBUF; output bf16, cast to f32 on host.

Sharding: core = (image b = core//2, row half core%2), 8-row patch-aligned
halo; weights replicated.
"""

import numpy as np
import ml_dtypes

import concourse.bass as bass
import concourse.mybir as mybir
from concourse import bacc, tile
from concourse.bass_utils import run_bass_kernel_spmd

BF16 = mybir.dt.bfloat16
F32 = mybir.dt.float32

DIM = 64
C2 = 340
P = 8
B, H, W = 4, 256, 256
N_CORES = 8
ROWS = H // 2
HALO = P
NCOL = 352          # 22 groups x 16, incl 12 pad channels
NGRP = 22
NPAIR = 11          # gate pairs (x1 group 2m, x2 group 2m+1)
RH = ROWS + 2 * HALO            # 144
NPR = RH // P                   # 18 patch rows
NBLK = NPR * 16                 # 288 blocks (2 patches each)
RING = 48                       # z ring rows
NWIN = 10                       # dwconv windows (14 valid rows each)
WSTEP = 14

_cache = {}


# ----------------------------------------------------------------- host math

def _build_basis():
    rows = []
    seen = set()
    p1, p2 = np.meshgrid(np.arange(P), np.arange(P), indexing="ij")
    for u in range(P):
        for v in range(P):
            if (u, v) in seen:
                continue
            nu, nv = (-u) % P, (-v) % P
            th = 2 * np.pi * (u * p1 + v * p2) / P
            if (nu, nv) == (u, v):
                rows.append((np.cos(th) / 8.0).ravel())
            else:
                seen.add((nu, nv))
                rows.append((np.sqrt(2) / 8.0) * np.cos(th).ravel())
                rows.append((np.sqrt(2) / 8.0) * np.sin(th).ravel())
            seen.add((u, v))
    return np.array(rows, dtype=np.float64)


def _lam_for(fft_w, C):
    basis = C.reshape(64, P, P)
    F = np.fft.rfft2(basis)
    w = fft_w.reshape(C2, 1, P, P // 2 + 1).astype(np.float64)
    r = np.fft.irfft2(F[None] * w, s=(P, P))
    return np.einsum('kpq,ckpq->ck', basis, r)      # [C2, 64]


def _col_to_c2():
    cols = np.full(NCOL, -1, np.int64)
    for g in range(NGRP):
        m, half = divmod(g, 2)
        for j in range(16):
            ch = 16 * m + j
            if ch < 170:
                cols[g * 16 + j] = ch + 170 * half
    return cols


def _pix_maps():
    C = _build_basis()
    CCrhs = np.zeros((128, 128))
    for pc2 in range(2):
        for f in range(64):
            for p1 in range(P):
                for p2 in range(P):
                    k = p1 * 16 + pc2 * 8 + p2
                    CCrhs[pc2 * 64 + f, k] = C[f, p1 * 8 + p2]
    return CCrhs.T.copy(), CCrhs, C


def _prep_weights(w_in, w_dw, fft_w, w_out):
    CClhsT, CCrhs, C = _pix_maps()
    lam = _lam_for(fft_w, C)
    cols = _col_to_c2()
    valid = cols >= 0

    w_inT = np.zeros((64, NCOL))
    w_inT[:, valid] = w_in.T[:, cols[valid]]
    w_in2 = np.concatenate([w_inT, w_inT], axis=0)          # [128, NCOL]

    lam_t = np.zeros((128, NCOL))
    lam_sel = np.zeros((NCOL, 64))
    lam_sel[valid] = lam[cols[valid]]
    lam_t[:] = np.tile(lam_sel.T, (2, 1))[:128]

    dw = w_dw.reshape(C2, 3, 3)
    # dwconv banded lhsT per (subgroup of 8 ch, dx): [128, 128]
    # in partition (c,i) = c*16+i, out partition (t,c) = t*8+c (t-major so
    # the valid rows t<14 are a contiguous partition prefix),
    # T[(c,i),(t,c)] = dw[ch, i-t, dx] for i-t in {0,1,2}, t < 14
    dd = np.zeros((128, 44 * 3 * 128))
    for sg in range(44):
        for dx in range(3):
            blk = np.zeros((128, 128))
            for c in range(8):
                c2 = cols[sg * 8 + c]
                if c2 < 0:
                    continue
                for t in range(WSTEP):
                    for dy in range(3):
                        i = t + dy
                        if i < 16:
                            blk[c * 16 + i, t * 8 + c] = dw[c2, dy, dx]
            dd[:, (sg * 3 + dx) * 128:(sg * 3 + dx + 1) * 128] = blk

    # project_out lhsT chunks: gate channel gch = 16m+8s+c
    wo0 = np.zeros((128, 64))
    wo0[0:128] = w_out.T[0:128]
    wo1 = np.zeros((48, 64))
    wo1[0:42] = w_out.T[128:170]
    bf = ml_dtypes.bfloat16
    return {
        "w_in2": w_in2.astype(bf),
        "cclhsT": CClhsT.astype(bf),
        "ccrhs": CCrhs.astype(bf),
        "lam_t": lam_t.astype(bf),
        "dd": dd.astype(bf),
        "wo0": wo0.astype(bf),
        "wo1": wo1.astype(bf),
    }


# ---------------------------------------------------------------- bass build

def build_nc(dbg=False):
    G = mybir.ActivationFunctionType.Gelu
    chunks = [(0, 128), (128, 128), (256, 96)]
    # gather-ready patch row (emission point) per window, and deadline checks
    ready = {4: [0], 6: [1], 8: [2], 9: [3], 11: [4], 13: [5],
             15: [6], 16: [7], 17: [8, 9]}

    nc = bacc.Bacc("TRN2", target_bir_lowering=False, debug=False,
                   num_devices=N_CORES)
    x_d = nc.dram_tensor("x", [128, NPR * 16 * 64], BF16, kind="ExternalInput")
    win2_d = nc.dram_tensor("w_in2", [128, NCOL], BF16, kind="ExternalInput")
    cclhsT_d = nc.dram_tensor("cclhsT", [128, 128], BF16, kind="ExternalInput")
    ccrhs_d = nc.dram_tensor("ccrhs", [128, 128], BF16, kind="ExternalInput")
    lam_d = nc.dram_tensor("lam_t", [128, NCOL], BF16, kind="ExternalInput")
    dd_d = nc.dram_tensor("dd", [128, 44 * 3 * 128], BF16,
                          kind="ExternalInput")
    wo0_d = nc.dram_tensor("wo0", [128, 64], BF16, kind="ExternalInput")
    wo1_d = nc.dram_tensor("wo1", [48, 64], BF16, kind="ExternalInput")
    out_d = nc.dram_tensor("out", [64, ROWS * W], BF16, kind="ExternalOutput")
    # z bounce: row index = (ci, c_sub, g), cols = (row, w)
    zkind = "ExternalOutput" if dbg else "Internal"
    zd = nc.dram_tensor("zd", [3 * 8 * 16, RH * W], BF16, kind=zkind)
    gd = nc.dram_tensor("gd", [128, NWIN * WSTEP * W], BF16, kind=zkind)
    gtd = nc.dram_tensor("gtd", [128, 512], BF16, kind=zkind)
    gbuf = nc.dram_tensor("gbuf", [176, ROWS * W], BF16, kind="Internal")

    with tile.TileContext(nc) as tc:
        with tc.tile_pool(name="consts", bufs=1) as cpool:
            w_in2 = cpool.tile([128, NCOL], BF16)
            nc.sync.dma_start(out=w_in2[:], in_=win2_d[:])
            cclhsT = cpool.tile([128, 128], BF16)
            nc.sync.dma_start(out=cclhsT[:], in_=cclhsT_d[:])
            ccrhs = cpool.tile([128, 128], BF16)
            nc.sync.dma_start(out=ccrhs[:], in_=ccrhs_d[:])
            lam_t = cpool.tile([128, NCOL], BF16)
            nc.sync.dma_start(out=lam_t[:], in_=lam_d[:])
            dd = cpool.tile([128, 44 * 3 * 128], BF16)
            nc.sync.dma_start(out=dd[:], in_=dd_d[:])
            wo0 = cpool.tile([128, 64], BF16)
            nc.sync.dma_start(out=wo0[:], in_=wo0_d[:])
            wo1 = cpool.tile([48, 64], BF16)
            nc.sync.dma_start(out=wo1[:], in_=wo1_d[:])
            zd_v = zd[:].rearrange("(ci c g) (r w) -> ci c g r w",
                                   ci=3, c=8, g=16, r=RH)

            with (
                tc.tile_pool(name="xs", bufs=6) as xpool,
                tc.tile_pool(name="zs", bufs=2) as zspool,
                tc.tile_pool(name="sf", bufs=6) as sfpool,
                tc.tile_pool(name="sb", bufs=8) as sbpool,
                tc.tile_pool(name="zt", bufs=2) as ztpool,
                tc.tile_pool(name="ge", bufs=6) as gepool,
                tc.tile_pool(name="gt", bufs=6) as gtpool,
                tc.tile_pool(name="gp", bufs=2) as gppool,
                tc.tile_pool(name="ob", bufs=1) as obpool,
                tc.tile_pool(name="fb", bufs=2, space="PSUM") as fbpool,
                tc.tile_pool(name="pz", bufs=2, space="PSUM") as pzpool,
                tc.tile_pool(name="pq", bufs=4, space="PSUM") as pqpool,
            ):
                # x loads, per patch-row: [128, 16 blocks * 64 ch]
                xts = {}

                def load_x(p):
                    xt = xpool.tile([128, 16 * 64], BF16, tag="x")
                    nc.sync.dma_start(out=xt[:],
                                      in_=x_d[:, p * 1024:(p + 1) * 1024])
                    xts[p] = xt

                for p in range(4):
                    load_x(p)

                # HAM warmup while DMAs land
                wps = fbpool.tile([128, NCOL], F32, tag="fb", name="warm")
                for _ in range(40):
                    nc.tensor.matmul(wps[:, 0:128], cclhsT[:], ccrhs[:],
                                     start=True, stop=True)

                def emit_a(p):
                    """Stage A for patch row p: 16 blocks -> z rows in DRAM."""
                    xt = xts.pop(p)
                    zs = zspool.tile([128, 3 * 8 * W], BF16, tag="zs")
                    zs_v = zs[:].rearrange("c (ci r w) -> c ci r w",
                                           ci=3, r=8)
                    for pq in range(4):
                        sBs = []
                        for k in range(2):          # pairs in group
                            kp = pq * 2 + k
                            psF = fbpool.tile([128, 128], F32, tag="fb",
                                              name="psF")
                            nc.tensor.matmul(psF[:],
                                             xt[:, kp * 128:(kp + 1) * 128],
                                             cclhsT[:], start=True, stop=True)
                            sF = sfpool.tile([128, 128], BF16, tag="sF")
                            nc.scalar.copy(sF[:], psF[:])
                            for b in range(2):
                                psB = fbpool.tile([128, NCOL], F32, tag="fb",
                                                  name="psB")
                                nc.tensor.matmul(psB[:],
                                                 sF[b * 64:(b + 1) * 64, :],
                                                 w_in2[b * 64:(b + 1) * 64, :],
                                                 start=True, stop=True)
                                sB = sbpool.tile([128, NCOL], BF16, tag="sB")
                                nc.vector.tensor_mul(sB[:], psB[:], lam_t[:])
                                sBs.append(sB)
                        for ci, (c0, m) in enumerate(chunks):
                            psZ = pzpool.tile([128, 512], F32, tag="pz")
                            for pj in range(4):
                                nc.tensor.matmul(
                                    psZ[0:m, pj * 128:(pj + 1) * 128],
                                    sBs[pj][:, c0:c0 + m], ccrhs[:],
                                    start=(pj == 0), stop=(pj == 3))
                            src = psZ[0:m, :].rearrange(
                                "c (pj p1 q) -> c p1 pj q", pj=4, p1=8)
                            dst = zs_v[0:m, ci, :,
                                       pq * 64:(pq + 1) * 64].rearrange(
                                "c r (pj q) -> c r pj q", pj=4)
                            if ci == 2:
                                nc.vector.tensor_copy(dst, src)
                            else:
                                nc.scalar.copy(dst, src)
                    # write z rows to DRAM: one DMA per chunk
                    # zd row layout: (c_sub, ci, g)
                    zdr = zd[:].rearrange("(c ci g) x -> ci g c x",
                                          ci=3, c=8, g=16)
                    for ci, (c0, m) in enumerate(chunks):
                        ng = m // 8
                        nc.sync.dma_start(
                            out=zdr[ci, 0:ng, :,
                                    p * 8 * W:(p + 1) * 8 * W],
                            in_=zs[0:m, ci * 8 * W:(ci + 1) * 8 * W])

                def emit_gather(w):
                    """Prefetch z window w from DRAM into one zt tile.

                    zt cols = (ci*16+g, w); one DMA per (chunk, c_sub).
                    """
                    zr0 = WSTEP * w + 7                  # first z row needed
                    nrow = min(16, RH - zr0)
                    zdg = zd[:].rearrange("(c ci g) (r w) -> ci c r g w",
                                          c=8, ci=3, g=16, r=RH)
                    zt = ztpool.tile([128, 48 * W], BF16, tag="zt")
                    ztv = zt[:].rearrange("p (ci g w) -> ci p g w",
                                          ci=3, g=16)
                    for ci, (c0, m) in enumerate(chunks):
                        ng = m // 8
                        for cs in range(8):
                            dst = ztv[ci, cs * 16:cs * 16 + nrow, 0:ng, :]
                            eng = (nc.sync, nc.gpsimd, nc.scalar)[cs % 3]
                            eng.dma_start(
                                out=dst,
                                in_=zdg[ci, cs, zr0:zr0 + nrow, 0:ng, :])
                    return zt

                def emit_proj(w, gps):
                    """project_out for window w from restructured gate chs."""
                    gp0, gp1 = gps
                    nv = min(WSTEP, ROWS - WSTEP * w)
                    ncols = nv * W
                    ob = obpool.tile([64, WSTEP * W], BF16, tag="ob")
                    nct = (ncols + 511) // 512
                    for ct in range(nct):
                        c0_, c1 = ct * 512, min((ct + 1) * 512, ncols)
                        po = pqpool.tile([64, 512], F32, tag="q", name="po")
                        nc.tensor.matmul(po[:, 0:c1 - c0_], wo0[:],
                                         gp0[:, c0_:c1], start=True,
                                         stop=False)
                        nc.tensor.matmul(po[:, 0:c1 - c0_], wo1[:],
                                         gp1[0:48, c0_:c1], start=False,
                                         stop=True)
                        nc.vector.tensor_copy(ob[:, c0_:c1],
                                              po[:, 0:c1 - c0_])
                    nc.sync.dma_start(
                        out=out_d[:, WSTEP * w * W:WSTEP * w * W + ncols],
                        in_=ob[:, 0:ncols])

                def emit_b(w, zt):
                    """Stage B for window w: dwconv + gate; returns gp tiles."""
                    nv = min(WSTEP, ROWS - WSTEP * w)    # valid out rows
                    gp0 = gppool.tile([128, WSTEP * W], BF16, tag="gp0")
                    gp1 = gppool.tile([48, WSTEP * W], BF16, tag="gp1")
                    dxs = [(1, 0, 0, 256), (0, 0, 1, 255), (2, 1, 0, 255)]
                    for m_ in range(NPAIR):
                        ci = m_ // 4
                        goff = (m_ % 4) * 4             # subgroup offset
                        pss = []
                        for half in range(2):
                            ps = pqpool.tile([128, 512], F32, tag="q",
                                             name=f"q{half}")
                            for s in range(2):
                                sg = 16 * ci + goff + 2 * half + s
                                gl = ci * 16 + goff + 2 * half + s
                                for dx, wi0, wo0_, wn in dxs:
                                    lhs = dd[:, (sg * 3 + dx) * 128:
                                             (sg * 3 + dx + 1) * 128]
                                    rhs = zt[:, gl * 256 + wi0:
                                             gl * 256 + wi0 + wn]
                                    nc.tensor.matmul(
                                        ps[:, s * 256 + wo0_:
                                           s * 256 + wo0_ + wn],
                                        lhs, rhs,
                                        start=(dx == 1), stop=(dx == 2))
                            pss.append(ps)
                        ge = gepool.tile([128, 512], BF16, tag="ge")
                        nc.scalar.activation(ge[:], pss[0][:], G)
                        gt = gtpool.tile([128, 512], BF16, tag="gt")
                        nc.vector.tensor_mul(gt[:], ge[:], pss[1][:])
                        if dbg and w == 1 and m_ == 0:
                            nc.sync.dma_start(out=gtd[:], in_=gt[:])
                        # restructure: (c,t) partitions -> gate-ch partitions
                        gbv = gbuf[:].rearrange("gc (r w) -> r gc w", w=W)
                        for s_ in range(2):
                            src = gt[0:nv * 8, s_ * 256:(s_ + 1) * 256]
                            gc0 = 16 * m_ + 8 * s_
                            dst = gbv[WSTEP * w:WSTEP * w + nv,
                                      gc0:gc0 + 8, :]
                            eng = nc.gpsimd if s_ == 0 else nc.sync
                            eng.dma_start(out=dst, in_=src)
                    # load restructured gate channels back from DRAM
                    nc.gpsimd.dma_start(
                        out=gp0[:, 0:nv * W],
                        in_=gbuf[0:128, WSTEP * w * W:(WSTEP * w + nv) * W])
                    nc.sync.dma_start(
                        out=gp1[:, 0:nv * W],
                        in_=gbuf[128:176, WSTEP * w * W:(WSTEP * w + nv) * W])
                    if dbg:
                        nc.sync.dma_start(
                            out=gd[:, w * WSTEP * W:w * WSTEP * W + nv * W],
                            in_=gp0[:, 0:nv * W])
                    return gp0, gp1

                # software-pipelined schedule: gathers one step ahead of
                # dwconv+gate, proj one step behind
                pend_g = {}          # w -> zts
                pend_p = {}          # w -> gp tiles

                def step_b(p):
                    for w in list(pend_p):
                        emit_proj(w, pend_p.pop(w))
                    for w in list(pend_g):
                        pend_p[w] = emit_b(w, pend_g.pop(w))
                    for w in ready.get(p, []):
                        pend_g[w] = emit_gather(w)

                for p in range(NPR):
                    if p + 4 < NPR:
                        load_x(p + 4)
                    emit_a(p)
                    step_b(p)
                for w in list(pend_g):
                    pend_p[w] = emit_b(w, pend_g.pop(w))
                for w in list(pend_p):
                    emit_proj(w, pend_p.pop(w))

    nc.compile()
    return nc


# ----------------------------------------------------------------- interface

def _get_program(dbg=False):
    key = ("nc", dbg)
    if key not in _cache:
        _cache[key] = build_nc(dbg)
    return _cache[key]


def _shard_x(x):
    """Per core: pixel-major blocks [128, (pr, blk16, ch64)]."""
    shards = []
    for c in range(N_CORES):
        b, hh = divmod(c, 2)
        r0 = hh * ROWS
        xs = np.zeros((DIM, RH, W), np.float32)
        lo, hi = r0 - HALO, r0 + ROWS + HALO
        slo, shi = max(lo, 0), min(hi, x.shape[2])
        xs[:, slo - lo:shi - lo] = x[b, :, slo:shi]
        # [c, pr, p1, pcp, pc2, p2] -> [(p1 pc2 p2), (pr pcp c)]
        xp = xs.reshape(DIM, NPR, P, 16, 2, P).transpose(2, 4, 5, 1, 3, 0)
        shards.append(np.ascontiguousarray(xp).reshape(128, NPR * 16 * 64)
                      .astype(ml_dtypes.bfloat16))
    return shards


def _run(x, w_in, w_dw, fft_w, w_out, trace=False, dbg=False):
    nc = _get_program(dbg)
    wts = _prep_weights(np.asarray(w_in, np.float32),
                        np.asarray(w_dw, np.float32).reshape(C2, 3, 3),
                        np.asarray(fft_w, np.float32),
                        np.asarray(w_out, np.float32))
    shards = _shard_x(np.asarray(x, np.float32))
    in_maps = [{"x": s, **wts} for s in shards]
    res = run_bass_kernel_spmd(nc, in_maps, core_ids=list(range(N_CORES)),
                               trace=trace)
    out = np.zeros((B, DIM, H, W), np.float32)
    for c in range(N_CORES):
        b, hh = divmod(c, 2)
        out[b, :, hh * ROWS:(hh + 1) * ROWS] = (
            res.results[c]["out"].astype(np.float32).reshape(DIM, ROWS, W))
    if dbg:
        return out, res.exec_time_ns, res.results
    return out, res.exec_time_ns


def kernel(x, w_in, w_dw, fft_w, w_out):
    out, _ = _run(x, w_in, w_dw, fft_w, w_out, trace=False)
    return out
